# revision 1
# baseline (speedup 1.0000x reference)
"""Trainium2 Bass kernel for a dense transformer decoder block, 8-core SPMD.

Sharding: sequence-parallel. Core c owns token rows [256c:256c+256) of each
batch (512 rows total). Each core computes QKV for its own rows, the K/V
shards are AllGathered on-device, attention runs dense over all keys with a
host-supplied 0/1 mask (uniform program on every core), and the wo / MLP
parts are fully row-local. Matmuls run in bf16 with fp32 accumulation;
softmax and the residual stream stay fp32.
"""
import math
from contextlib import ExitStack

import numpy as np
import ml_dtypes

import concourse.bacc as bacc
import concourse.bass as bass
import concourse.tile as tile
import concourse.mybir as mybir
from concourse.bass_utils import run_bass_kernel_spmd
from concourse.masks import make_identity

AF = mybir.ActivationFunctionType
BF = mybir.dt.bfloat16
F32 = mybir.dt.float32

N_CORES = 8
P = 128
B, S, D, H, HD, DFF = 2, 2048, 2048, 16, 128, 8192
GRP = 4                   # cores per batch (AllGather subgroup size)
C = S // GRP              # 512 tokens per core, contiguous rows of one batch
TOK = C                   # owned tokens per core
NT = TOK // P             # 4 token tiles
KD = D // P               # 16 contraction tiles over D
NDF = DFF // P            # 64 dff tiles
NC_ = D // 512            # 4 output 512-chunks
EPS = 1e-8
KBLK = H * P * TOK        # elements in the K region of one AG block
BLK = 2 * KBLK            # elements per AG block (K + V)
NKV = S // P              # 16 kv tiles over the core's batch


def _emit(nc, collective=True, upto=99, reps=1):
    """Emit the whole per-core program inside a TileContext."""
    x_in = nc.dram_tensor("x_own", [TOK, D], F32, kind="ExternalInput")
    wq_in = nc.dram_tensor("wqt", [H, P, KD, HD], BF, kind="ExternalInput")
    wk_in = nc.dram_tensor("wkt", [H, P, KD, HD], BF, kind="ExternalInput")
    wv_in = nc.dram_tensor("wvT", [D, D], BF, kind="ExternalInput")
    wo_in = nc.dram_tensor("woT", [D, D], BF, kind="ExternalInput")
    f1_in = nc.dram_tensor("fc1t", [NDF, P, KD, HD], BF, kind="ExternalInput")
    f2_in = nc.dram_tensor("fc2T", [DFF, D], BF, kind="ExternalInput")
    f1b_in = nc.dram_tensor("fc1b", [P, NDF], F32, kind="ExternalInput")
    f2b_in = nc.dram_tensor("fc2b", [D], F32, kind="ExternalInput")
    msk_in = nc.dram_tensor("mask", [NKV, P, C], BF, kind="ExternalInput")
    y_out = nc.dram_tensor("y", [TOK, D], F32, kind="ExternalOutput")

    xv = x_in.ap().rearrange("(t p) d -> p t d", p=P)     # [P, NT, D] DRAM view
    yv = y_out.ap().rearrange("(t p) d -> p t d", p=P)

    with tile.TileContext(nc) as tc, ExitStack() as ctx:
        singles = ctx.enter_context(tc.tile_pool(name="singles", bufs=1))
        persist = ctx.enter_context(tc.tile_pool(name="persist", bufs=1))
        scratch = ctx.enter_context(tc.tile_pool(name="scratch", bufs=2))
        dram = ctx.enter_context(tc.tile_pool(name="dram", bufs=1, space="DRAM"))

        ident = singles.tile([P, P], BF, tag="ident")
        make_identity(nc, ident)
        ones_bf = singles.tile([P, 1], BF, tag="ones")
        nc.vector.memset(ones_bf, 1.0)
        f2b_bc = singles.tile([P, D], F32, tag="f2b")
        f2b_ap = f2b_in.ap()
        nc.gpsimd.dma_start(
            out=f2b_bc,
            in_=bass.AP(tensor=f2b_ap.tensor, offset=f2b_ap.offset, ap=[[0, P], [1, D]]),
        )
        f1b_sb = singles.tile([P, NDF], F32, tag="f1b")
        nc.sync.dma_start(f1b_sb, f1b_in[:])

        BN_STATS_DIM = nc.vector.BN_STATS_DIM
        BN_AGGR_DIM = nc.vector.BN_AGGR_DIM
        NSUB = D // nc.vector.BN_STATS_FMAX

        if reps > 1:
            assert not collective, "hardware loop cannot contain a collective"
            ctx.enter_context(tc.For_i(0, reps, 1))

        def norm_and_transpose(src_view, pp):
            """RMS-norm each token tile of the fp32 DRAM view [P, NT, D],
            then PE-transpose into nT [P, KD, TOK] bf16 (feature-major)."""
            nT = persist.tile([P, KD, TOK], BF, tag="nT", name="nT")
            for mt in range(NT):
                xt = scratch.tile([P, D], F32, tag="xt", name="xt")
                nc.sync.dma_start(xt, src_view[:, mt])
                stats = scratch.tile([P, NSUB, BN_STATS_DIM], F32, tag="bst",
                                     name="stats")
                x4 = xt.rearrange("p (s f) -> p s f", s=NSUB)
                for sg in range(NSUB):
                    nc.vector.bn_stats(out=stats[:, sg], in_=x4[:, sg])
                mv = scratch.tile([P, BN_AGGR_DIM], F32, tag="bag", name="mv")
                nc.vector.bn_aggr(out=mv, in_=stats)
                msq = scratch.tile([P, 1], F32, tag="msq", name="msq")
                nc.vector.tensor_mul(msq, mv[:, 0:1], mv[:, 0:1])
                nc.vector.tensor_add(msq, msq, mv[:, 1:2])   # mean(x^2)
                lnv = scratch.tile([P, 1], F32, tag="lnv", name="lnv")
                nc.scalar.activation(out=lnv, in_=msq, func=AF.Ln)
                rms = scratch.tile([P, 1], F32, tag="rms", name="rms")
                nc.scalar.activation(out=rms, in_=lnv, func=AF.Exp, scale=0.5)
                nc.vector.tensor_scalar_add(rms, rms, EPS)
                rinv = scratch.tile([P, 1], F32, tag="rinv", name="rinv")
                nc.vector.reciprocal(rinv, rms)
                nbf = scratch.tile([P, D], BF, tag="nbf", name="nbf")
                nc.vector.tensor_scalar_mul(nbf, xt, rinv)
                for kt in range(KD):
                    ps = pp.tile([P, P], BF, tag="acc", name="tps")
                    nc.tensor.transpose(ps, nbf[:, kt * P:(kt + 1) * P], ident)
                    nc.vector.tensor_copy(
                        out=nT[:, kt, mt * P:(mt + 1) * P], in_=ps)
            return nT

        kv_in = dram.tile([BLK], BF, tag="kv_in", name="kv_in")
        kvK = kv_in[0:KBLK].rearrange("(h p t) -> h p t", h=H, p=P)
        kvV = kv_in[KBLK:BLK].rearrange("(h t d) -> h t d", h=H, t=TOK)
        kout_d = dram.tile([GRP * KBLK], BF, tag="kout_d", name="kout_d")
        vout_d = dram.tile([GRP * KBLK], BF, tag="vout_d", name="vout_d")
        x2d = dram.tile([P, NT, D], F32, tag="x2d", name="x2d")

        qa_ctx = ExitStack()
        qa = qa_ctx.enter_context(tc.tile_pool(name="qa", bufs=1))
        qt = qa.tile([P, H, TOK], BF, tag="qt", name="qt")

        with tc.tile_pool(name="pp", bufs=4, space="PSUM") as pp, \
             tc.tile_pool(name="wqk_pool", bufs=3) as wqk_pool, \
             tc.tile_pool(name="wv_pool", bufs=3) as wv_pool:
            n1T = norm_and_transpose(xv, pp)
            if upto < 2:
                return

            # Q and K projections, head-major transposed layout [hd, tok]
            for h in range(H):
                wqsb = wqk_pool.tile([P, KD, HD], BF, tag="wqk", name="wqsb")
                nc.sync.dma_start(wqsb, wq_in[h])
                psq = pp.tile([P, TOK], F32, tag="acc", name="psq")
                for kt in range(KD):
                    nc.tensor.matmul(psq, lhsT=wqsb[:, kt], rhs=n1T[:, kt],
                                     start=(kt == 0), stop=(kt == KD - 1))
                nc.vector.tensor_copy(out=qt[:, h], in_=psq)

                wksb = wqk_pool.tile([P, KD, HD], BF, tag="wqk", name="wksb")
                nc.sync.dma_start(wksb, wk_in[h])
                psk = pp.tile([P, TOK], F32, tag="acc", name="psk")
                for kt in range(KD):
                    nc.tensor.matmul(psk, lhsT=wksb[:, kt], rhs=n1T[:, kt],
                                     start=(kt == 0), stop=(kt == KD - 1))
                ksb = scratch.tile([P, TOK], BF, tag="kout", name="ksb")
                nc.vector.tensor_copy(out=ksb, in_=psk)
                nc.sync.dma_start(kvK[h], ksb)

            # V projection, natural layout [tok, d], written head-major
            for n in range(NC_):
                pss = [pp.tile([P, 512], F32, tag="acc", name=f"psv{mt}")
                       for mt in range(NT)]
                for kt in range(KD):
                    wvsb = wv_pool.tile([P, 512], BF, tag="wv", name="wvsb")
                    nc.sync.dma_start(
                        wvsb, wv_in[kt * P:(kt + 1) * P, n * 512:(n + 1) * 512])
                    for mt in range(NT):
                        nc.tensor.matmul(pss[mt],
                                         lhsT=n1T[:, kt, mt * P:(mt + 1) * P],
                                         rhs=wvsb,
                                         start=(kt == 0), stop=(kt == KD - 1))
                for mt in range(NT):
                    vsb = scratch.tile([P, 512], BF, tag="vout", name="vsb")
                    nc.vector.tensor_copy(out=vsb, in_=pss[mt])
                    for hh in range(4):
                        h = n * 4 + hh
                        nc.sync.dma_start(
                            kvV[h, mt * P:(mt + 1) * P, :],
                            vsb[:, hh * P:(hh + 1) * P])

        if upto < 3:
            return
        if collective:
            nc.gpsimd.collective_compute(
                "AllGather",
                mybir.AluOpType.bypass,
                replica_groups=[[0, 1, 2, 3], [4, 5, 6, 7]],
                ins=[kv_in[0:KBLK].opt()],
                outs=[kout_d.opt()],
            )
            nc.gpsimd.collective_compute(
                "AllGather",
                mybir.AluOpType.bypass,
                replica_groups=[[0, 1, 2, 3], [4, 5, 6, 7]],
                ins=[kv_in[KBLK:BLK].opt()],
                outs=[vout_d.opt()],
            )
        else:
            # timing-only variant: fake the AllGathers with local copies that
            # mimic their HBM write traffic (and avoid garbage in the outputs)
            for r in range(GRP):
                nc.sync.dma_start(kout_d[r * KBLK:(r + 1) * KBLK], kv_in[0:KBLK])
                nc.sync.dma_start(vout_d[r * KBLK:(r + 1) * KBLK], kv_in[KBLK:BLK])

        avt = qa.tile([P, H, TOK], BF, tag="avt", name="avt")
        if upto < 4:
            qa_ctx.close()
            return

        # attention: dense over the 16 key tiles of this core's batch
        with tc.tile_pool(name="sc_ps", bufs=2, space="PSUM") as sc_ps, \
             tc.tile_pool(name="av_ps", bufs=2, space="PSUM") as av_ps, \
             tc.tile_pool(name="dn_ps", bufs=2, space="PSUM") as dn_ps, \
             tc.tile_pool(name="kt_pool", bufs=2) as kt_pool, \
             tc.tile_pool(name="vf_pool", bufs=2) as vf_pool, \
             tc.tile_pool(name="ex_pool", bufs=3) as ex_pool, \
             tc.tile_pool(name="bi_pool", bufs=2) as bi_pool, \
             tc.tile_pool(name="msk_pool", bufs=1) as msk_pool:
            msk = msk_pool.tile([P, NKV, C], BF, tag="msk", name="msk")
            nc.sync.dma_start(msk, msk_in.ap().rearrange("j p q -> p j q"))
            for h in range(H):
                ktf = kt_pool.tile([P, GRP, TOK], BF, tag="ktf", name="ktf")
                vf = vf_pool.tile([P, GRP, 4, HD], BF, tag="vf", name="vf")
                for r in range(GRP):
                    blkK = kout_d[r * KBLK:(r + 1) * KBLK].rearrange(
                        "(h p t) -> h p t", h=H, p=P)
                    blkV = vout_d[r * KBLK:(r + 1) * KBLK].rearrange(
                        "(h t d) -> h t d", h=H, t=TOK)
                    nc.sync.dma_start(ktf[:, r], blkK[h])
                    nc.sync.dma_start(
                        vf[:, r], blkV[h].rearrange("(q p) d -> p q d", p=P))
                qv = qt[:, h]
                dn = dn_ps.tile([1, C], F32, tag="dn", name="dn")
                av = av_ps.tile([P, C], F32, tag="av", name="av")
                prev = None
                for jj in range(NKV // 2):
                    sc = sc_ps.tile([P, 2, C], F32, tag="sc", name="sc")
                    for u in range(2):
                        j = jj * 2 + u
                        ksrc = ktf[:, j // 4, (j % 4) * P:(j % 4) * P + P]
                        nc.tensor.matmul(sc[:, u], lhsT=ksrc, rhs=qv,
                                         start=True, stop=True)
                    ex = ex_pool.tile([P, 2, C], BF, tag="ex", name="ex")
                    nc.scalar.activation(out=ex, in_=sc, func=AF.Exp)
                    nc.vector.tensor_mul(ex, ex, msk[:, jj * 2:(jj + 1) * 2, :])
                    if prev is not None:
                        pex, pjj = prev
                        for u in range(2):
                            j = pjj * 2 + u
                            nc.tensor.matmul(dn, lhsT=ones_bf, rhs=pex[:, u],
                                             start=(j == 0), stop=False)
                            vsrc = vf[:, j // 4, j % 4, :]
                            nc.tensor.matmul(av, lhsT=vsrc, rhs=pex[:, u],
                                             start=(j == 0), stop=False)
                    prev = (ex, jj)
                pex, pjj = prev
                for u in range(2):
                    j = pjj * 2 + u
                    nc.tensor.matmul(dn, lhsT=ones_bf, rhs=pex[:, u],
                                     start=False, stop=(j == NKV - 1))
                    vsrc = vf[:, j // 4, j % 4, :]
                    nc.tensor.matmul(av, lhsT=vsrc, rhs=pex[:, u],
                                     start=False, stop=(j == NKV - 1))
                inv = bi_pool.tile([1, C], F32, tag="inv", name="inv")
                nc.vector.reciprocal(inv, dn)
                bi = bi_pool.tile([P, C], F32, tag="bi", name="bi")
                nc.gpsimd.partition_broadcast(bi, inv)
                nc.vector.tensor_tensor(
                    out=avt[:, h], in0=av, in1=bi,
                    op=mybir.AluOpType.mult)

        with tc.tile_pool(name="wops", bufs=4, space="PSUM") as wops, \
             tc.tile_pool(name="wo_pool", bufs=3) as wo_pool:
            # wo projection + residual -> x2 (DRAM)
            for n in range(NC_):
                pss = [wops.tile([P, 512], F32, tag="acc", name=f"pso{mt}")
                       for mt in range(NT)]
                for h in range(H):
                    wosb = wo_pool.tile([P, 512], BF, tag="wo", name="wosb")
                    nc.sync.dma_start(
                        wosb, wo_in[h * P:(h + 1) * P, n * 512:(n + 1) * 512])
                    for mt in range(NT):
                        nc.tensor.matmul(pss[mt],
                                         lhsT=avt[:, h, mt * P:(mt + 1) * P],
                                         rhs=wosb,
                                         start=(h == 0), stop=(h == H - 1))
                for mt in range(NT):
                    xre = scratch.tile([P, 512], F32, tag="xre", name="xre")
                    nc.sync.dma_start(xre, xv[:, mt, n * 512:(n + 1) * 512])
                    x2sb = scratch.tile([P, 512], F32, tag="x2sb", name="x2sb")
                    nc.vector.tensor_add(out=x2sb, in0=pss[mt], in1=xre)
                    nc.sync.dma_start(x2d[:, mt, n * 512:(n + 1) * 512], x2sb)
        qa_ctx.close()

        if upto < 6:
            return
        with tc.tile_pool(name="mm", bufs=4, space="PSUM") as mm, \
             tc.tile_pool(name="f1_pool", bufs=2) as f1_pool, \
             tc.tile_pool(name="f2_pool", bufs=2) as f2_pool:
            n3T = norm_and_transpose(x2d, mm)
            if upto < 7:
                return

            # fc1 + silu -> hT [dff, tok] bf16
            with tc.tile_pool(name="ht_pool", bufs=1) as ht_pool:
                hT = ht_pool.tile([P, NDF, TOK], BF, tag="hT", name="hT")
                for dt in range(NDF):
                    wsb = f1_pool.tile([P, KD, HD], BF, tag="f1", name="f1sb")
                    nc.sync.dma_start(wsb, f1_in[dt])
                    ps = mm.tile([P, TOK], F32, tag="acc", name="psf1")
                    for kt in range(KD):
                        nc.tensor.matmul(ps, lhsT=wsb[:, kt], rhs=n3T[:, kt],
                                         start=(kt == 0), stop=(kt == KD - 1))
                    nc.scalar.activation(out=hT[:, dt], in_=ps, func=AF.Silu,
                                         bias=f1b_sb[:, dt:dt + 1], scale=1.0)

                if upto < 8:
                    return
                # fc2 + bias + residual -> y
                for n in range(NC_):
                    pss = [mm.tile([P, 512], F32, tag="acc", name=f"psf2{mt}")
                           for mt in range(NT)]
                    for half in range(2):
                        HD2 = NDF // 2
                        wsb = f2_pool.tile([P, HD2, 512], BF, tag="f2",
                                           name="f2sb")
                        nc.sync.dma_start(
                            wsb,
                            f2_in.ap().rearrange(
                                "(o p) d -> p o d", p=P)[
                                :, half * HD2:(half + 1) * HD2,
                                n * 512:(n + 1) * 512])
                        for do in range(HD2):
                            dt = half * HD2 + do
                            for mt in range(NT):
                                nc.tensor.matmul(pss[mt],
                                                 lhsT=hT[:, dt, mt * P:(mt + 1) * P],
                                                 rhs=wsb[:, do],
                                                 start=(dt == 0),
                                                 stop=(dt == NDF - 1))
                    for mt in range(NT):
                        x2re = scratch.tile([P, 512], F32, tag="x2re", name="x2re")
                        nc.sync.dma_start(x2re, x2d[:, mt, n * 512:(n + 1) * 512])
                        osb = scratch.tile([P, 512], F32, tag="osb", name="osb")
                        nc.vector.tensor_add(osb, pss[mt], x2re)
                        nc.vector.tensor_add(osb, osb,
                                             f2b_bc[:, n * 512:(n + 1) * 512])
                        nc.sync.dma_start(yv[:, mt, n * 512:(n + 1) * 512], osb)


def build_program(collective=True, upto=99, reps=1):
    nc = bacc.Bacc("TRN2", target_bir_lowering=False, debug=False,
                   num_devices=N_CORES)
    _emit(nc, collective=collective, upto=upto, reps=reps)
    nc.finalize()
    return nc


def _bf(x):
    return np.ascontiguousarray(x.astype(ml_dtypes.bfloat16))


def prep_inputs(inputs):
    """Host-side prep: fold alpha/scale into weights, build per-core in_maps."""
    x = np.asarray(inputs["x"], dtype=np.float32)
    tgt = np.asarray(inputs["tgt_mask"])
    wq = np.asarray(inputs["wq"], dtype=np.float32)
    wk = np.asarray(inputs["wk"], dtype=np.float32)
    wv = np.asarray(inputs["wv"], dtype=np.float32)
    wo = np.asarray(inputs["wo"], dtype=np.float32)
    a1 = np.asarray(inputs["alpha1"], dtype=np.float32)
    a3 = np.asarray(inputs["alpha3"], dtype=np.float32)
    f1w = np.asarray(inputs["fc1_w"], dtype=np.float32)
    f1b = np.asarray(inputs["fc1_b"], dtype=np.float32)
    f2w = np.asarray(inputs["fc2_w"], dtype=np.float32)
    f2b = np.asarray(inputs["fc2_b"], dtype=np.float32)

    wqT = (wq * a1[None, :] / math.sqrt(HD)).T          # [D_in, D_out]
    wkT = (wk * a1[None, :]).T
    wvT = (wv * a1[None, :]).T
    woT = wo.T
    f1T = (f1w * a3[None, :]).T                          # [D, DFF]
    f2T = f2w.T                                          # [DFF, D]

    # stationary pre-tiling: [out_tile, partition(k), k_tile, out_sub]
    wqt = _bf(wqT.reshape(KD, P, H, HD).transpose(2, 1, 0, 3))
    wkt = _bf(wkT.reshape(KD, P, H, HD).transpose(2, 1, 0, 3))
    f1t = _bf(f1T.reshape(KD, P, NDF, HD).transpose(2, 1, 0, 3))
    wvT_b = _bf(wvT)
    woT_b = _bf(woT)
    f2T_b = _bf(f2T)
    f1b_t = np.ascontiguousarray(f1b.reshape(NDF, P).T.astype(np.float32))

    tm = np.asarray(tgt[0, 0], dtype=np.float32)         # [S, S]
    in_maps = []
    for c in range(N_CORES):
        b = c // GRP
        rows = slice((c % GRP) * C, (c % GRP + 1) * C)
        x_own = np.ascontiguousarray(x[b, rows])
        mask = _bf(np.ascontiguousarray(
            tm[rows, :].T.reshape(NKV, P, C)))
        in_maps.append({
            "x_own": x_own,
            "wqt": wqt, "wkt": wkt, "wvT": wvT_b, "woT": woT_b,
            "fc1t": f1t, "fc2T": f2T_b,
            "fc1b": f1b_t, "fc2b": f2b,
            "mask": mask,
        })
    return in_maps


def assemble_output(results):
    y = np.empty((B, S, D), dtype=np.float32)
    for c in range(N_CORES):
        yc = results[c]["y"]                              # [TOK, D]
        y[c // GRP, (c % GRP) * C:(c % GRP + 1) * C] = yc
    return y


_CACHE = {}


def kernel(**inputs) -> np.ndarray:
    if "nc" not in _CACHE:
        _CACHE["nc"] = build_program()
    nc = _CACHE["nc"]
    in_maps = prep_inputs(inputs)
    res = run_bass_kernel_spmd(nc, in_maps, core_ids=list(range(N_CORES)))
    return assemble_output(res.results)



# revision 16
# speedup vs baseline: 1.1095x; 1.1095x over previous
"""Trainium2 Bass kernel for a dense transformer decoder block, 8-core SPMD.

Sharding: sequence-parallel. Core c owns token rows [512c:512c+512) of one
batch (GRP=4 cores per batch). Each core computes QKV for its own rows; K and
V shards are AllGathered within the 4-core group; attention runs dense over
all keys with a host-supplied 0/1 mask; wo / MLP are row-local.

Scheduling is built around two facts about TRN2:
 - the PE clock ramps (0.65/1.2/2.4 GHz) and only reaches full rate after
   ~3us of gapless execution, so the emission order keeps matmuls dense;
 - collectives serialize on one stream at ~63 GB/s, so they are chunked
   (K half / V full / K half) and overlapped with the projection loops:
   K(h0-7) -> AG(K0) -> V -> AG(V) -> K(h8-15) -> AG(K1) -> Q -> attention.

Matmuls run in bf16 with fp32 accumulation; softmax + residual stay fp32.
"""
import math
from contextlib import ExitStack

import numpy as np
import ml_dtypes

import concourse.bacc as bacc
import concourse.bass as bass
import concourse.tile as tile
import concourse.mybir as mybir
from concourse.bass_utils import run_bass_kernel_spmd
from concourse.masks import make_identity

AF = mybir.ActivationFunctionType
BF = mybir.dt.bfloat16
F32 = mybir.dt.float32

N_CORES = 8
P = 128
B, S, D, H, HD, DFF = 2, 2048, 2048, 16, 128, 8192
GRP = 4                   # cores per batch (AllGather subgroup size)
C = S // GRP              # 512 tokens per core, contiguous rows of one batch
TOK = C
NT = TOK // P             # 4 token tiles
KD = D // P               # 16 contraction tiles over D
NDF = DFF // P            # 64 dff tiles
NC_ = D // 512            # 4 output 512-chunks
NKV = S // P              # 16 kv tiles over the core's batch
HH = H // 2               # 8 heads per K AllGather chunk
EPS = 1e-8

RG = [[0, 1, 2, 3], [4, 5, 6, 7]]


def _emit(nc):
    x_in = nc.dram_tensor("x_own", [TOK, D], F32, kind="ExternalInput")
    wq_in = nc.dram_tensor("wqt", [H, P, KD, HD], BF, kind="ExternalInput")
    wk_in = nc.dram_tensor("wkt", [H, P, KD, HD], BF, kind="ExternalInput")
    wv_in = nc.dram_tensor("wvt", [P, KD, D], BF, kind="ExternalInput")
    wo_in = nc.dram_tensor("wot", [NC_, P, H, 512], BF, kind="ExternalInput")
    f1_in = nc.dram_tensor("fc1t", [NDF, P, KD, HD], BF, kind="ExternalInput")
    f2_in = nc.dram_tensor("fc2t", [NC_, 4, P, 16, 512], BF, kind="ExternalInput")
    f1b_in = nc.dram_tensor("fc1b", [P, NDF], F32, kind="ExternalInput")
    f2b_in = nc.dram_tensor("fc2b", [D], F32, kind="ExternalInput")
    msk_in = nc.dram_tensor("mask", [NKV, P, C], BF, kind="ExternalInput")
    y_out = nc.dram_tensor("y", [TOK, D], F32, kind="ExternalOutput")

    xv = x_in.ap().rearrange("(t p) d -> p t d", p=P)     # [P, NT, D] DRAM view
    yv = y_out.ap().rearrange("(t p) d -> p t d", p=P)

    with tile.TileContext(nc) as tc, ExitStack() as ctx:
        singles = ctx.enter_context(tc.tile_pool(name="singles", bufs=1))
        persist = ctx.enter_context(tc.tile_pool(name="persist", bufs=1))
        scratch = ctx.enter_context(tc.tile_pool(name="scratch", bufs=2))
        dram = ctx.enter_context(tc.tile_pool(name="dram", bufs=1, space="DRAM"))

        ident = singles.tile([P, P], BF, tag="ident")
        make_identity(nc, ident)
        ones_bf = singles.tile([P, 1], BF, tag="ones")
        nc.vector.memset(ones_bf, 1.0)
        f1b_sb = singles.tile([P, NDF], F32, tag="f1b")
        nc.sync.dma_start(f1b_sb, f1b_in[:])

        BN_STATS_DIM = nc.vector.BN_STATS_DIM
        BN_AGGR_DIM = nc.vector.BN_AGGR_DIM
        NSUB = D // nc.vector.BN_STATS_FMAX

        def norm_and_transpose(get_tile, pp):
            """RMS-norm each token tile ([P, D] fp32, from get_tile(mt)), then
            PE-transpose into nT [P, KD, TOK] bf16 (feature-major)."""
            nT = persist.tile([P, KD, TOK], BF, tag="nT", name="nT")
            for mt in range(NT):
                xt = get_tile(mt)
                stats = scratch.tile([P, NSUB, BN_STATS_DIM], F32, tag="bst",
                                     name="stats")
                x4 = xt.rearrange("p (s f) -> p s f", s=NSUB)
                for sg in range(NSUB):
                    nc.vector.bn_stats(out=stats[:, sg], in_=x4[:, sg])
                mv = scratch.tile([P, BN_AGGR_DIM], F32, tag="bag", name="mv")
                nc.vector.bn_aggr(out=mv, in_=stats)
                msq = scratch.tile([P, 1], F32, tag="msq", name="msq")
                nc.vector.tensor_mul(msq, mv[:, 0:1], mv[:, 0:1])
                nc.vector.tensor_add(msq, msq, mv[:, 1:2])   # mean(x^2)
                lnv = scratch.tile([P, 1], F32, tag="lnv", name="lnv")
                nc.scalar.activation(out=lnv, in_=msq, func=AF.Ln)
                rms = scratch.tile([P, 1], F32, tag="rms", name="rms")
                nc.scalar.activation(out=rms, in_=lnv, func=AF.Exp, scale=0.5)
                nc.vector.tensor_scalar_add(rms, rms, EPS)
                rinv = scratch.tile([P, 1], F32, tag="rinv", name="rinv")
                nc.vector.reciprocal(rinv, rms)
                nbf = scratch.tile([P, D], BF, tag="nbf", name="nbf")
                nc.vector.tensor_scalar_mul(nbf, xt, rinv)
                for kt in range(KD):
                    ps = pp.tile([P, P], BF, tag="tp", name="tps")
                    nc.tensor.transpose(ps, nbf[:, kt * P:(kt + 1) * P], ident)
                    nc.vector.tensor_copy(
                        out=nT[:, kt, mt * P:(mt + 1) * P], in_=ps)
            return nT

        # DRAM staging for the collectives (1-D tiles + shaped views)
        KSZ = P * HH * TOK
        VSZ = NT * P * D
        kin = [dram.tile([KSZ], BF, tag=f"kin{i}", name=f"kin{i}")
               for i in range(2)]
        kout = [dram.tile([GRP * KSZ], BF, tag=f"kout{i}", name=f"kout{i}")
                for i in range(2)]
        vin = dram.tile([VSZ], BF, tag="vin", name="vin")
        vout = dram.tile([GRP * VSZ], BF, tag="vout", name="vout")
        kinv = [t[:].rearrange("(p h t) -> p h t", p=P, h=HH) for t in kin]
        koutv = [t[:].rearrange("(r p h t) -> r p h t", r=GRP, p=P, h=HH)
                 for t in kout]
        vinv = vin[:].rearrange("(m p d) -> m p d", m=NT, p=P)
        voutv = vout[:].rearrange("(r m p d) -> r m p d", r=GRP, m=NT, p=P)

        x2d = dram.tile([P, NT, D], F32, tag="x2d", name="x2d")

        # mask for attention: pool outlives phase 1 so the DMA issues early
        msk_ctx = ExitStack()
        mskp = msk_ctx.enter_context(tc.tile_pool(name="mskp", bufs=1))
        msk = mskp.tile([P, NKV, C], BF, tag="msk", name="msk")
        nc.sync.dma_start(msk, msk_in.ap().rearrange("j p q -> p j q"))

        qa_ctx = ExitStack()
        qa = qa_ctx.enter_context(tc.tile_pool(name="qa", bufs=1))
        qt = qa.tile([P, H, TOK], BF, tag="qt", name="qt")
        avt = qa.tile([P, H, TOK], BF, tag="avt", name="avt")

        # ---- phase 1: norm1 + K/V/Q projections, AllGathers interleaved ----
        with tc.tile_pool(name="pp", bufs=4, space="PSUM") as pp, \
             tc.tile_pool(name="wres", bufs=1) as wres, \
             tc.tile_pool(name="wqk_pool", bufs=3) as wqk_pool, \
             tc.tile_pool(name="kst_pool", bufs=2) as kst_pool, \
             tc.tile_pool(name="vst_pool", bufs=2) as vst_pool:
            wv_sb = wres.tile([P, KD, D], BF, tag="wv", name="wv_sb")
            nc.sync.dma_start(wv_sb, wv_in[:])

            def load_x(mt):
                xt = scratch.tile([P, D], F32, tag="xt", name="xt")
                nc.sync.dma_start(xt, xv[:, mt])
                return xt

            n1T = norm_and_transpose(load_x, pp)

            def k_half(half):
                kst = kst_pool.tile([P, HH, TOK], BF, tag="kst", name="kst")
                for hh in range(HH):
                    h = half * HH + hh
                    wksb = wqk_pool.tile([P, KD, HD], BF, tag="wqk", name="wksb")
                    nc.sync.dma_start(wksb, wk_in[h])
                    psk = pp.tile([P, TOK], F32, tag="acc", name="psk")
                    for kt in range(KD):
                        nc.tensor.matmul(psk, lhsT=wksb[:, kt], rhs=n1T[:, kt],
                                         start=(kt == 0), stop=(kt == KD - 1))
                    nc.vector.tensor_copy(out=kst[:, hh], in_=psk)
                nc.sync.dma_start(kinv[half], kst)
                nc.gpsimd.collective_compute(
                    "AllGather", mybir.AluOpType.bypass, replica_groups=RG,
                    ins=[kin[half].opt()],
                    outs=[kout[half].opt()],
                )

            # K heads 0-7 -> AG(K0)
            k_half(0)

            # V (all tokens) -> AG(V)
            for mt in range(NT):
                vst = vst_pool.tile([P, D], BF, tag="vst", name="vst")
                for n in range(NC_):
                    psv = pp.tile([P, 512], F32, tag="acc", name="psv")
                    for kt in range(KD):
                        nc.tensor.matmul(
                            psv, lhsT=n1T[:, kt, mt * P:(mt + 1) * P],
                            rhs=wv_sb[:, kt, n * 512:(n + 1) * 512],
                            start=(kt == 0), stop=(kt == KD - 1))
                    nc.vector.tensor_copy(out=vst[:, n * 512:(n + 1) * 512],
                                          in_=psv)
                nc.sync.dma_start(vinv[mt], vst)
            nc.gpsimd.collective_compute(
                "AllGather", mybir.AluOpType.bypass, replica_groups=RG,
                ins=[vin.opt()],
                outs=[vout.opt()],
            )

            # K heads 8-15 -> AG(K1)
            k_half(1)

            # Q all heads
            for h in range(H):
                wqsb = wqk_pool.tile([P, KD, HD], BF, tag="wqk", name="wqsb")
                nc.sync.dma_start(wqsb, wq_in[h])
                psq = pp.tile([P, TOK], F32, tag="acc", name="psq")
                for kt in range(KD):
                    nc.tensor.matmul(psq, lhsT=wqsb[:, kt], rhs=n1T[:, kt],
                                     start=(kt == 0), stop=(kt == KD - 1))
                nc.vector.tensor_copy(out=qt[:, h], in_=psq)

        # ---- phase 2: attention ----
        with tc.tile_pool(name="sc_ps", bufs=2, space="PSUM") as sc_ps, \
             tc.tile_pool(name="av_ps", bufs=2, space="PSUM") as av_ps, \
             tc.tile_pool(name="dn_ps", bufs=2, space="PSUM") as dn_ps, \
             tc.tile_pool(name="kt_pool", bufs=3) as kt_pool, \
             tc.tile_pool(name="vall_pool", bufs=1) as vall_pool, \
             tc.tile_pool(name="ex_pool", bufs=3) as ex_pool, \
             tc.tile_pool(name="bi_pool", bufs=2) as bi_pool:
            # V for all heads, resident: one tile per gather block r so the
            # first heads only wait on block r=0's loads
            vall = [vall_pool.tile([P, NT, D], BF, tag=f"vall{r}",
                                   name=f"vall{r}") for r in range(GRP)]
            for r in range(GRP):
                for mt in range(NT):
                    nc.sync.dma_start(vall[r][:, mt], voutv[r, mt])

            for h in range(H):
                katt = kt_pool.tile([P, GRP, TOK], BF, tag="katt", name="katt")
                src = koutv[h // HH]
                nc.sync.dma_start(
                    katt, src.rearrange("r p h t -> p r h t")[:, :, h % HH])
                qv = qt[:, h]
                dn = dn_ps.tile([1, C], F32, tag="dn", name="dn")
                av = av_ps.tile([P, C], F32, tag="av", name="av")
                prev = None
                for jj in range(NKV // 2):
                    sc = sc_ps.tile([P, 2, C], F32, tag="sc", name="sc")
                    for u in range(2):
                        j = jj * 2 + u
                        ksrc = katt[:, j // 4, (j % 4) * P:(j % 4) * P + P]
                        nc.tensor.matmul(sc[:, u], lhsT=ksrc, rhs=qv,
                                         start=True, stop=True)
                    ex = ex_pool.tile([P, 2, C], BF, tag="ex", name="ex")
                    nc.scalar.activation(out=ex, in_=sc, func=AF.Exp)
                    nc.vector.tensor_mul(ex, ex, msk[:, jj * 2:(jj + 1) * 2, :])
                    if prev is not None:
                        pex, pjj = prev
                        for u in range(2):
                            j = pjj * 2 + u
                            nc.tensor.matmul(dn, lhsT=ones_bf, rhs=pex[:, u],
                                             start=(j == 0), stop=False)
                            vsrc = vall[j // 4][:, j % 4, h * P:(h + 1) * P]
                            nc.tensor.matmul(av, lhsT=vsrc, rhs=pex[:, u],
                                             start=(j == 0), stop=False)
                    prev = (ex, jj)
                pex, pjj = prev
                for u in range(2):
                    j = pjj * 2 + u
                    nc.tensor.matmul(dn, lhsT=ones_bf, rhs=pex[:, u],
                                     start=False, stop=(j == NKV - 1))
                    vsrc = vall[j // 4][:, j % 4, h * P:(h + 1) * P]
                    nc.tensor.matmul(av, lhsT=vsrc, rhs=pex[:, u],
                                     start=False, stop=(j == NKV - 1))
                inv = bi_pool.tile([1, C], F32, tag="inv", name="inv")
                nc.vector.reciprocal(inv, dn)
                bi = bi_pool.tile([P, C], F32, tag="bi", name="bi")
                nc.gpsimd.partition_broadcast(bi, inv)
                nc.vector.tensor_tensor(
                    out=avt[:, h], in0=av, in1=bi,
                    op=mybir.AluOpType.mult)

        # ---- phase 3: wo projection + residual (x2 stays in SBUF) ----
        with tc.tile_pool(name="wops", bufs=4, space="PSUM") as wops, \
             tc.tile_pool(name="wo_pool", bufs=2) as wo_pool:
            for n in range(NC_):
                won = wo_pool.tile([P, H, 512], BF, tag="wo", name="won")
                nc.sync.dma_start(won, wo_in[n])
                pss = [wops.tile([P, 512], F32, tag="acc", name=f"pso{mt}")
                       for mt in range(NT)]
                for h in range(H):
                    for mt in range(NT):
                        nc.tensor.matmul(pss[mt],
                                         lhsT=avt[:, h, mt * P:(mt + 1) * P],
                                         rhs=won[:, h],
                                         start=(h == 0), stop=(h == H - 1))
                for mt in range(NT):
                    xre = scratch.tile([P, 512], F32, tag="xre", name="xre")
                    nc.sync.dma_start(xre, xv[:, mt, n * 512:(n + 1) * 512])
                    x2sb = scratch.tile([P, 512], F32, tag="x2sb", name="x2sb")
                    nc.vector.tensor_add(out=x2sb, in0=pss[mt], in1=xre)
                    nc.sync.dma_start(x2d[:, mt, n * 512:(n + 1) * 512], x2sb)
        qa_ctx.close()
        msk_ctx.close()

        # ---- phase 4: norm3 + fc1 + fc2 ----
        with tc.tile_pool(name="mm", bufs=4, space="PSUM") as mm, \
             tc.tile_pool(name="f1_pool", bufs=3) as f1_pool, \
             tc.tile_pool(name="f2_pool", bufs=2) as f2_pool, \
             tc.tile_pool(name="f2bp", bufs=1) as f2bp:
            f2b_bc = f2bp.tile([P, D], F32, tag="f2b", name="f2b")
            f2b_ap = f2b_in.ap()
            nc.gpsimd.dma_start(
                out=f2b_bc,
                in_=bass.AP(tensor=f2b_ap.tensor, offset=f2b_ap.offset,
                            ap=[[0, P], [1, D]]),
            )

            def load_x2(mt):
                xt = scratch.tile([P, D], F32, tag="xt", name="xt")
                nc.sync.dma_start(xt, x2d[:, mt])
                return xt

            n3T = norm_and_transpose(load_x2, mm)

            with tc.tile_pool(name="ht_pool", bufs=1) as ht_pool:
                hT = ht_pool.tile([P, NDF, TOK], BF, tag="hT", name="hT")
                for dt in range(NDF):
                    wsb = f1_pool.tile([P, KD, HD], BF, tag="f1", name="f1sb")
                    nc.sync.dma_start(wsb, f1_in[dt])
                    ps = mm.tile([P, TOK], F32, tag="acc", name="psf1")
                    for kt in range(KD):
                        nc.tensor.matmul(ps, lhsT=wsb[:, kt], rhs=n3T[:, kt],
                                         start=(kt == 0), stop=(kt == KD - 1))
                    nc.scalar.activation(out=hT[:, dt], in_=ps, func=AF.Silu,
                                         bias=f1b_sb[:, dt:dt + 1], scale=1.0)

                # fc2 + bias + residual -> y
                for n in range(NC_):
                    pss = [mm.tile([P, 512], F32, tag="acc", name=f"psf2{mt}")
                           for mt in range(NT)]
                    for oh in range(4):
                        wsb = f2_pool.tile([P, 16, 512], BF, tag="f2",
                                           name="f2sb")
                        nc.sync.dma_start(wsb, f2_in[n, oh])
                        for o in range(16):
                            dt = oh * 16 + o
                            for mt in range(NT):
                                nc.tensor.matmul(
                                    pss[mt],
                                    lhsT=hT[:, dt, mt * P:(mt + 1) * P],
                                    rhs=wsb[:, o],
                                    start=(dt == 0), stop=(dt == NDF - 1))
                    for mt in range(NT):
                        x2re = scratch.tile([P, 512], F32, tag="x2re",
                                            name="x2re")
                        nc.sync.dma_start(x2re,
                                          x2d[:, mt, n * 512:(n + 1) * 512])
                        osb = scratch.tile([P, 512], F32, tag="osb", name="osb")
                        nc.vector.tensor_add(osb, pss[mt], x2re)
                        nc.vector.tensor_add(osb, osb,
                                             f2b_bc[:, n * 512:(n + 1) * 512])
                        nc.sync.dma_start(yv[:, mt, n * 512:(n + 1) * 512], osb)


def build_program():
    nc = bacc.Bacc("TRN2", target_bir_lowering=False, debug=False,
                   num_devices=N_CORES)
    _emit(nc)
    nc.finalize()
    return nc


def _bf(x):
    return np.ascontiguousarray(x.astype(ml_dtypes.bfloat16))


def prep_inputs(inputs):
    """Host-side prep: fold alpha/scale into weights, build per-core in_maps."""
    x = np.asarray(inputs["x"], dtype=np.float32)
    tgt = np.asarray(inputs["tgt_mask"])
    wq = np.asarray(inputs["wq"], dtype=np.float32)
    wk = np.asarray(inputs["wk"], dtype=np.float32)
    wv = np.asarray(inputs["wv"], dtype=np.float32)
    wo = np.asarray(inputs["wo"], dtype=np.float32)
    a1 = np.asarray(inputs["alpha1"], dtype=np.float32)
    a3 = np.asarray(inputs["alpha3"], dtype=np.float32)
    f1w = np.asarray(inputs["fc1_w"], dtype=np.float32)
    f1b = np.asarray(inputs["fc1_b"], dtype=np.float32)
    f2w = np.asarray(inputs["fc2_w"], dtype=np.float32)
    f2b = np.asarray(inputs["fc2_b"], dtype=np.float32)

    wqT = (wq * a1[None, :] / math.sqrt(HD)).T          # [D_in, D_out]
    wkT = (wk * a1[None, :]).T
    wvT = (wv * a1[None, :]).T
    woT = wo.T
    f1T = (f1w * a3[None, :]).T                          # [D, DFF]
    f2T = f2w.T                                          # [DFF, D]

    # stationary pre-tiling: [out_tile, partition(k), k_tile, out_sub]
    wqt = _bf(wqT.reshape(KD, P, H, HD).transpose(2, 1, 0, 3))
    wkt = _bf(wkT.reshape(KD, P, H, HD).transpose(2, 1, 0, 3))
    f1t = _bf(f1T.reshape(KD, P, NDF, HD).transpose(2, 1, 0, 3))
    # moving-weight pre-tiling, contiguous per partition line:
    wvt = _bf(wvT.reshape(KD, P, D).transpose(1, 0, 2))          # [P, KD, D]
    wot = _bf(woT.reshape(H, P, NC_, 512).transpose(2, 1, 0, 3))  # [n,p,h,512]
    f2t = _bf(f2T.reshape(4, 16, P, NC_, 512).transpose(3, 0, 2, 1, 4))
    f1b_t = np.ascontiguousarray(f1b.reshape(NDF, P).T.astype(np.float32))

    tm = np.asarray(tgt[0, 0], dtype=np.float32)         # [S, S]
    in_maps = []
    for c in range(N_CORES):
        b = c // GRP
        rows = slice((c % GRP) * C, (c % GRP + 1) * C)
        x_own = np.ascontiguousarray(x[b, rows])
        mask = _bf(np.ascontiguousarray(
            tm[rows, :].T.reshape(NKV, P, C)))
        in_maps.append({
            "x_own": x_own,
            "wqt": wqt, "wkt": wkt, "wvt": wvt, "wot": wot,
            "fc1t": f1t, "fc2t": f2t,
            "fc1b": f1b_t, "fc2b": f2b,
            "mask": mask,
        })
    return in_maps


def assemble_output(results):
    y = np.empty((B, S, D), dtype=np.float32)
    for c in range(N_CORES):
        yc = results[c]["y"]                              # [TOK, D]
        y[c // GRP, (c % GRP) * C:(c % GRP + 1) * C] = yc
    return y


_CACHE = {}


def kernel(**inputs) -> np.ndarray:
    if "nc" not in _CACHE:
        _CACHE["nc"] = build_program()
    nc = _CACHE["nc"]
    in_maps = prep_inputs(inputs)
    res = run_bass_kernel_spmd(nc, in_maps, core_ids=list(range(N_CORES)))
    return assemble_output(res.results)


# revision 23
# speedup vs baseline: 1.1160x; 1.0059x over previous
"""Trainium2 Bass kernel for a dense transformer decoder block, 8-core SPMD.

Sharding: sequence-parallel. Core c owns token rows [512c:512c+512) of one
batch (GRP=4 cores per batch). Each core computes QKV for its own rows; K and
V shards are AllGathered within the 4-core group; attention runs dense over
all keys with a host-supplied 0/1 mask; wo / MLP are row-local.

Scheduling is built around two facts about TRN2:
 - the PE clock ramps (0.65/1.2/2.4 GHz) and only reaches full rate after
   ~3us of gapless execution, so the emission order keeps matmuls dense;
 - collectives serialize on one stream at ~63 GB/s, so they are chunked
   (K half / V full / K half) and overlapped with the projection loops:
   K(h0-7) -> AG(K0) -> V -> AG(V) -> K(h8-15) -> AG(K1) -> Q -> attention.

Matmuls run in bf16 with fp32 accumulation; softmax + residual stay fp32.
"""
import math
from contextlib import ExitStack

import numpy as np
import ml_dtypes

import concourse.bacc as bacc
import concourse.bass as bass
import concourse.tile as tile
import concourse.mybir as mybir
from concourse.bass_utils import run_bass_kernel_spmd
from concourse.masks import make_identity

AF = mybir.ActivationFunctionType
BF = mybir.dt.bfloat16
F32 = mybir.dt.float32

N_CORES = 8
P = 128
B, S, D, H, HD, DFF = 2, 2048, 2048, 16, 128, 8192
GRP = 4                   # cores per batch (AllGather subgroup size)
C = S // GRP              # 512 tokens per core, contiguous rows of one batch
TOK = C
NT = TOK // P             # 4 token tiles
KD = D // P               # 16 contraction tiles over D
NDF = DFF // P            # 64 dff tiles
NC_ = D // 512            # 4 output 512-chunks
NKV = S // P              # 16 kv tiles over the core's batch
HH = H // 2               # 8 heads per K AllGather chunk
EPS = 1e-8

RG = [[0, 1, 2, 3], [4, 5, 6, 7]]


def _emit(nc):
    x_in = nc.dram_tensor("x_own", [TOK, D], F32, kind="ExternalInput")
    wq_in = nc.dram_tensor("wqt", [H, P, KD, HD], BF, kind="ExternalInput")
    wk_in = nc.dram_tensor("wkt", [H, P, KD, HD], BF, kind="ExternalInput")
    wv_in = nc.dram_tensor("wvt", [P, KD, D], BF, kind="ExternalInput")
    wo_in = nc.dram_tensor("wot", [NC_, P, H, 512], BF, kind="ExternalInput")
    f1_in = nc.dram_tensor("fc1t", [NDF, P, KD, HD], BF, kind="ExternalInput")
    f2_in = nc.dram_tensor("fc2t", [NC_, 4, P, 16, 512], BF, kind="ExternalInput")
    f1b_in = nc.dram_tensor("fc1b", [P, NDF], F32, kind="ExternalInput")
    f2b_in = nc.dram_tensor("fc2b", [D], F32, kind="ExternalInput")
    msk_in = nc.dram_tensor("mask", [NKV, P, C], BF, kind="ExternalInput")
    y_out = nc.dram_tensor("y", [TOK, D], F32, kind="ExternalOutput")

    xv = x_in.ap().rearrange("(t p) d -> p t d", p=P)     # [P, NT, D] DRAM view
    yv = y_out.ap().rearrange("(t p) d -> p t d", p=P)

    with tile.TileContext(nc) as tc, ExitStack() as ctx:
        singles = ctx.enter_context(tc.tile_pool(name="singles", bufs=1))
        persist = ctx.enter_context(tc.tile_pool(name="persist", bufs=1))
        scratch = ctx.enter_context(tc.tile_pool(name="scratch", bufs=2))
        dram = ctx.enter_context(tc.tile_pool(name="dram", bufs=1, space="DRAM"))

        ident = singles.tile([P, P], BF, tag="ident")
        make_identity(nc, ident)
        ones_bf = singles.tile([P, 1], BF, tag="ones")
        nc.vector.memset(ones_bf, 1.0)
        f1b_sb = singles.tile([P, NDF], F32, tag="f1b")
        nc.scalar.dma_start(f1b_sb, f1b_in[:])

        BN_STATS_DIM = nc.vector.BN_STATS_DIM
        BN_AGGR_DIM = nc.vector.BN_AGGR_DIM
        NSUB = D // nc.vector.BN_STATS_FMAX

        def norm_and_transpose(get_tile, pp):
            """RMS-norm each token tile ([P, D] fp32, from get_tile(mt)), then
            PE-transpose into nT [P, KD, TOK] bf16 (feature-major)."""
            nT = persist.tile([P, KD, TOK], BF, tag="nT", name="nT")
            for mt in range(NT):
                xt = get_tile(mt)
                stats = scratch.tile([P, NSUB, BN_STATS_DIM], F32, tag="bst",
                                     name="stats")
                x4 = xt.rearrange("p (s f) -> p s f", s=NSUB)
                for sg in range(NSUB):
                    nc.vector.bn_stats(out=stats[:, sg], in_=x4[:, sg])
                mv = scratch.tile([P, BN_AGGR_DIM], F32, tag="bag", name="mv")
                nc.vector.bn_aggr(out=mv, in_=stats)
                msq = scratch.tile([P, 1], F32, tag="msq", name="msq")
                nc.vector.tensor_mul(msq, mv[:, 0:1], mv[:, 0:1])
                nc.vector.tensor_add(msq, msq, mv[:, 1:2])   # mean(x^2)
                lnv = scratch.tile([P, 1], F32, tag="lnv", name="lnv")
                nc.scalar.activation(out=lnv, in_=msq, func=AF.Ln)
                rms = scratch.tile([P, 1], F32, tag="rms", name="rms")
                nc.scalar.activation(out=rms, in_=lnv, func=AF.Exp, scale=0.5)
                nc.vector.tensor_scalar_add(rms, rms, EPS)
                rinv = scratch.tile([P, 1], F32, tag="rinv", name="rinv")
                nc.vector.reciprocal(rinv, rms)
                nbf = scratch.tile([P, D], BF, tag="nbf", name="nbf")
                nc.vector.tensor_scalar_mul(nbf, xt, rinv)
                for kt in range(KD):
                    ps = pp.tile([P, P], BF, tag="tp", name="tps")
                    nc.tensor.transpose(ps, nbf[:, kt * P:(kt + 1) * P], ident)
                    nc.vector.tensor_copy(
                        out=nT[:, kt, mt * P:(mt + 1) * P], in_=ps)
            return nT

        # DRAM staging for the collectives (1-D tiles + shaped views)
        KSZ = P * HH * TOK
        VSZ = NT * P * D
        kin = [dram.tile([KSZ], BF, tag=f"kin{i}", name=f"kin{i}")
               for i in range(2)]
        kout = [dram.tile([GRP * KSZ], BF, tag=f"kout{i}", name=f"kout{i}")
                for i in range(2)]
        vin = dram.tile([VSZ], BF, tag="vin", name="vin")
        vout = dram.tile([GRP * VSZ], BF, tag="vout", name="vout")
        kinv = [t[:].rearrange("(p h t) -> p h t", p=P, h=HH) for t in kin]
        koutv = [t[:].rearrange("(r p h t) -> r p h t", r=GRP, p=P, h=HH)
                 for t in kout]
        vinv = vin[:].rearrange("(m p d) -> m p d", m=NT, p=P)
        voutv = vout[:].rearrange("(r m p d) -> r m p d", r=GRP, m=NT, p=P)

        x2d = dram.tile([P, NT, D], F32, tag="x2d", name="x2d")

        # mask for attention: pool outlives phase 1 so the DMA issues early
        msk_ctx = ExitStack()
        mskp = msk_ctx.enter_context(tc.tile_pool(name="mskp", bufs=1))
        msk = mskp.tile([P, NKV, C], BF, tag="msk", name="msk")
        nc.scalar.dma_start(msk, msk_in.ap().rearrange("j p q -> p j q"))

        qa_ctx = ExitStack()
        qa = qa_ctx.enter_context(tc.tile_pool(name="qa", bufs=1))
        qt = qa.tile([P, H, TOK], BF, tag="qt", name="qt")
        avt = qa.tile([P, H, TOK], BF, tag="avt", name="avt")

        # ---- phase 1: norm1 + K/V/Q projections, AllGathers interleaved ----
        with tc.tile_pool(name="pp", bufs=4, space="PSUM") as pp, \
             tc.tile_pool(name="wres", bufs=1) as wres, \
             tc.tile_pool(name="wqk_pool", bufs=3) as wqk_pool, \
             tc.tile_pool(name="kst_pool", bufs=2) as kst_pool, \
             tc.tile_pool(name="vst_pool", bufs=2) as vst_pool:
            wv_sb = wres.tile([P, KD, D], BF, tag="wv", name="wv_sb")
            nc.scalar.dma_start(wv_sb, wv_in[:])

            def load_x(mt):
                xt = scratch.tile([P, D], F32, tag="xt", name="xt")
                nc.sync.dma_start(xt, xv[:, mt])
                return xt

            n1T = norm_and_transpose(load_x, pp)

            def k_half(half):
                kst = kst_pool.tile([P, HH, TOK], BF, tag="kst", name="kst")
                for hh in range(HH):
                    h = half * HH + hh
                    wksb = wqk_pool.tile([P, KD, HD], BF, tag="wqk", name="wksb")
                    nc.sync.dma_start(wksb, wk_in[h])
                    psk = pp.tile([P, TOK], F32, tag="acc", name="psk")
                    for kt in range(KD):
                        nc.tensor.matmul(psk, lhsT=wksb[:, kt], rhs=n1T[:, kt],
                                         start=(kt == 0), stop=(kt == KD - 1))
                    nc.vector.tensor_copy(out=kst[:, hh], in_=psk)
                nc.gpsimd.dma_start(kinv[half], kst)
                nc.gpsimd.collective_compute(
                    "AllGather", mybir.AluOpType.bypass, replica_groups=RG,
                    ins=[kin[half].opt()],
                    outs=[kout[half].opt()],
                )

            # K heads 0-7 -> AG(K0)
            k_half(0)

            # V (all tokens) -> AG(V)
            for mt in range(NT):
                vst = vst_pool.tile([P, D], BF, tag="vst", name="vst")
                for n in range(NC_):
                    psv = pp.tile([P, 512], F32, tag="acc", name="psv")
                    for kt in range(KD):
                        nc.tensor.matmul(
                            psv, lhsT=n1T[:, kt, mt * P:(mt + 1) * P],
                            rhs=wv_sb[:, kt, n * 512:(n + 1) * 512],
                            start=(kt == 0), stop=(kt == KD - 1))
                    nc.vector.tensor_copy(out=vst[:, n * 512:(n + 1) * 512],
                                          in_=psv)
                nc.gpsimd.dma_start(vinv[mt], vst)
            nc.gpsimd.collective_compute(
                "AllGather", mybir.AluOpType.bypass, replica_groups=RG,
                ins=[vin.opt()],
                outs=[vout.opt()],
            )

            # K heads 8-15 -> AG(K1)
            k_half(1)

            # Q all heads
            for h in range(H):
                wqsb = wqk_pool.tile([P, KD, HD], BF, tag="wqk", name="wqsb")
                nc.sync.dma_start(wqsb, wq_in[h])
                psq = pp.tile([P, TOK], F32, tag="acc", name="psq")
                for kt in range(KD):
                    nc.tensor.matmul(psq, lhsT=wqsb[:, kt], rhs=n1T[:, kt],
                                     start=(kt == 0), stop=(kt == KD - 1))
                nc.vector.tensor_copy(out=qt[:, h], in_=psq)

        # ---- phase 2: attention ----
        with tc.tile_pool(name="sc_ps", bufs=2, space="PSUM") as sc_ps, \
             tc.tile_pool(name="av_ps", bufs=2, space="PSUM") as av_ps, \
             tc.tile_pool(name="dn_ps", bufs=2, space="PSUM") as dn_ps, \
             tc.tile_pool(name="kt_pool", bufs=3) as kt_pool, \
             tc.tile_pool(name="vall_pool", bufs=1) as vall_pool, \
             tc.tile_pool(name="ex_pool", bufs=3) as ex_pool, \
             tc.tile_pool(name="bi_pool", bufs=2) as bi_pool:
            # V for all heads, resident: one tile per gather block r so the
            # first heads only wait on block r=0's loads
            vall = [vall_pool.tile([P, NT, D], BF, tag=f"vall{r}",
                                   name=f"vall{r}") for r in range(GRP)]
            for r in range(GRP):
                for mt in range(NT):
                    nc.sync.dma_start(vall[r][:, mt], voutv[r, mt])

            for h in range(H):
                katt = kt_pool.tile([P, GRP, TOK], BF, tag="katt", name="katt")
                src = koutv[h // HH]
                nc.sync.dma_start(
                    katt, src.rearrange("r p h t -> p r h t")[:, :, h % HH])
                qv = qt[:, h]
                dn = dn_ps.tile([1, C], F32, tag="dn", name="dn")
                av = av_ps.tile([P, C], F32, tag="av", name="av")
                prev = None
                for jj in range(NKV // 2):
                    sc = sc_ps.tile([P, 2, C], F32, tag="sc", name="sc")
                    for u in range(2):
                        j = jj * 2 + u
                        ksrc = katt[:, j // 4, (j % 4) * P:(j % 4) * P + P]
                        nc.tensor.matmul(sc[:, u], lhsT=ksrc, rhs=qv,
                                         start=True, stop=True)
                    ex = ex_pool.tile([P, 2, C], BF, tag="ex", name="ex")
                    nc.scalar.activation(out=ex, in_=sc, func=AF.Exp)
                    nc.vector.tensor_mul(ex, ex, msk[:, jj * 2:(jj + 1) * 2, :])
                    if prev is not None:
                        pex, pjj = prev
                        for u in range(2):
                            j = pjj * 2 + u
                            nc.tensor.matmul(dn, lhsT=ones_bf, rhs=pex[:, u],
                                             start=(j == 0), stop=False)
                            vsrc = vall[j // 4][:, j % 4, h * P:(h + 1) * P]
                            nc.tensor.matmul(av, lhsT=vsrc, rhs=pex[:, u],
                                             start=(j == 0), stop=False)
                    prev = (ex, jj)
                pex, pjj = prev
                for u in range(2):
                    j = pjj * 2 + u
                    nc.tensor.matmul(dn, lhsT=ones_bf, rhs=pex[:, u],
                                     start=False, stop=(j == NKV - 1))
                    vsrc = vall[j // 4][:, j % 4, h * P:(h + 1) * P]
                    nc.tensor.matmul(av, lhsT=vsrc, rhs=pex[:, u],
                                     start=False, stop=(j == NKV - 1))
                inv = bi_pool.tile([1, C], F32, tag="inv", name="inv")
                nc.vector.reciprocal(inv, dn)
                bi = bi_pool.tile([P, C], F32, tag="bi", name="bi")
                nc.gpsimd.partition_broadcast(bi, inv)
                nc.vector.tensor_tensor(
                    out=avt[:, h], in0=av, in1=bi,
                    op=mybir.AluOpType.mult)

        # ---- phase 3: wo projection + residual (x2 stays in SBUF) ----
        with tc.tile_pool(name="wops", bufs=4, space="PSUM") as wops, \
             tc.tile_pool(name="wo_pool", bufs=2) as wo_pool:
            for n in range(NC_):
                won = wo_pool.tile([P, H, 512], BF, tag="wo", name="won")
                nc.sync.dma_start(won, wo_in[n])
                pss = [wops.tile([P, 512], F32, tag="acc", name=f"pso{mt}")
                       for mt in range(NT)]
                for h in range(H):
                    for mt in range(NT):
                        nc.tensor.matmul(pss[mt],
                                         lhsT=avt[:, h, mt * P:(mt + 1) * P],
                                         rhs=won[:, h],
                                         start=(h == 0), stop=(h == H - 1))
                for mt in range(NT):
                    xre = scratch.tile([P, 512], F32, tag="xre", name="xre")
                    nc.sync.dma_start(xre, xv[:, mt, n * 512:(n + 1) * 512])
                    x2sb = scratch.tile([P, 512], F32, tag="x2sb", name="x2sb")
                    nc.vector.tensor_add(out=x2sb, in0=pss[mt], in1=xre)
                    nc.gpsimd.dma_start(x2d[:, mt, n * 512:(n + 1) * 512], x2sb)
        qa_ctx.close()
        msk_ctx.close()

        # ---- phase 4: norm3 + fc1 + fc2 ----
        with tc.tile_pool(name="mm", bufs=4, space="PSUM") as mm, \
             tc.tile_pool(name="f1_pool", bufs=3) as f1_pool, \
             tc.tile_pool(name="f2_pool", bufs=2) as f2_pool, \
             tc.tile_pool(name="f2bp", bufs=1) as f2bp:
            f2b_bc = f2bp.tile([P, D], F32, tag="f2b", name="f2b")
            f2b_ap = f2b_in.ap()
            nc.gpsimd.dma_start(
                out=f2b_bc,
                in_=bass.AP(tensor=f2b_ap.tensor, offset=f2b_ap.offset,
                            ap=[[0, P], [1, D]]),
            )

            def load_x2(mt):
                xt = scratch.tile([P, D], F32, tag="xt", name="xt")
                nc.sync.dma_start(xt, x2d[:, mt])
                return xt

            n3T = norm_and_transpose(load_x2, mm)

            with tc.tile_pool(name="ht_pool", bufs=1) as ht_pool:
                hT = ht_pool.tile([P, NDF, TOK], BF, tag="hT", name="hT")
                for dt in range(NDF):
                    wsb = f1_pool.tile([P, KD, HD], BF, tag="f1", name="f1sb")
                    nc.sync.dma_start(wsb, f1_in[dt])
                    ps = mm.tile([P, TOK], F32, tag="acc", name="psf1")
                    for kt in range(KD):
                        nc.tensor.matmul(ps, lhsT=wsb[:, kt], rhs=n3T[:, kt],
                                         start=(kt == 0), stop=(kt == KD - 1))
                    nc.scalar.activation(out=hT[:, dt], in_=ps, func=AF.Silu,
                                         bias=f1b_sb[:, dt:dt + 1], scale=1.0)

                # fc2 + bias + residual -> y
                for n in range(NC_):
                    pss = [mm.tile([P, 512], F32, tag="acc", name=f"psf2{mt}")
                           for mt in range(NT)]
                    for oh in range(4):
                        wsb = f2_pool.tile([P, 16, 512], BF, tag="f2",
                                           name="f2sb")
                        nc.sync.dma_start(wsb, f2_in[n, oh])
                        for o in range(16):
                            dt = oh * 16 + o
                            for mt in range(NT):
                                nc.tensor.matmul(
                                    pss[mt],
                                    lhsT=hT[:, dt, mt * P:(mt + 1) * P],
                                    rhs=wsb[:, o],
                                    start=(dt == 0), stop=(dt == NDF - 1))
                    for mt in range(NT):
                        x2re = scratch.tile([P, 512], F32, tag="x2re",
                                            name="x2re")
                        nc.sync.dma_start(x2re,
                                          x2d[:, mt, n * 512:(n + 1) * 512])
                        osb = scratch.tile([P, 512], F32, tag="osb", name="osb")
                        nc.vector.tensor_add(osb, pss[mt], x2re)
                        nc.vector.tensor_add(osb, osb,
                                             f2b_bc[:, n * 512:(n + 1) * 512])
                        nc.gpsimd.dma_start(yv[:, mt, n * 512:(n + 1) * 512], osb)


def build_program():
    nc = bacc.Bacc("TRN2", target_bir_lowering=False, debug=False,
                   num_devices=N_CORES)
    _emit(nc)
    nc.finalize()
    return nc


def _bf(x):
    return np.ascontiguousarray(x.astype(ml_dtypes.bfloat16))


def prep_inputs(inputs):
    """Host-side prep: fold alpha/scale into weights, build per-core in_maps."""
    x = np.asarray(inputs["x"], dtype=np.float32)
    tgt = np.asarray(inputs["tgt_mask"])
    wq = np.asarray(inputs["wq"], dtype=np.float32)
    wk = np.asarray(inputs["wk"], dtype=np.float32)
    wv = np.asarray(inputs["wv"], dtype=np.float32)
    wo = np.asarray(inputs["wo"], dtype=np.float32)
    a1 = np.asarray(inputs["alpha1"], dtype=np.float32)
    a3 = np.asarray(inputs["alpha3"], dtype=np.float32)
    f1w = np.asarray(inputs["fc1_w"], dtype=np.float32)
    f1b = np.asarray(inputs["fc1_b"], dtype=np.float32)
    f2w = np.asarray(inputs["fc2_w"], dtype=np.float32)
    f2b = np.asarray(inputs["fc2_b"], dtype=np.float32)

    wqT = (wq * a1[None, :] / math.sqrt(HD)).T          # [D_in, D_out]
    wkT = (wk * a1[None, :]).T
    wvT = (wv * a1[None, :]).T
    woT = wo.T
    f1T = (f1w * a3[None, :]).T                          # [D, DFF]
    f2T = f2w.T                                          # [DFF, D]

    # stationary pre-tiling: [out_tile, partition(k), k_tile, out_sub]
    wqt = _bf(wqT.reshape(KD, P, H, HD).transpose(2, 1, 0, 3))
    wkt = _bf(wkT.reshape(KD, P, H, HD).transpose(2, 1, 0, 3))
    f1t = _bf(f1T.reshape(KD, P, NDF, HD).transpose(2, 1, 0, 3))
    # moving-weight pre-tiling, contiguous per partition line:
    wvt = _bf(wvT.reshape(KD, P, D).transpose(1, 0, 2))          # [P, KD, D]
    wot = _bf(woT.reshape(H, P, NC_, 512).transpose(2, 1, 0, 3))  # [n,p,h,512]
    f2t = _bf(f2T.reshape(4, 16, P, NC_, 512).transpose(3, 0, 2, 1, 4))
    f1b_t = np.ascontiguousarray(f1b.reshape(NDF, P).T.astype(np.float32))

    tm = np.asarray(tgt[0, 0], dtype=np.float32)         # [S, S]
    in_maps = []
    for c in range(N_CORES):
        b = c // GRP
        rows = slice((c % GRP) * C, (c % GRP + 1) * C)
        x_own = np.ascontiguousarray(x[b, rows])
        mask = _bf(np.ascontiguousarray(
            tm[rows, :].T.reshape(NKV, P, C)))
        in_maps.append({
            "x_own": x_own,
            "wqt": wqt, "wkt": wkt, "wvt": wvt, "wot": wot,
            "fc1t": f1t, "fc2t": f2t,
            "fc1b": f1b_t, "fc2b": f2b,
            "mask": mask,
        })
    return in_maps


def assemble_output(results):
    y = np.empty((B, S, D), dtype=np.float32)
    for c in range(N_CORES):
        yc = results[c]["y"]                              # [TOK, D]
        y[c // GRP, (c % GRP) * C:(c % GRP + 1) * C] = yc
    return y


_CACHE = {}


def kernel(**inputs) -> np.ndarray:
    if "nc" not in _CACHE:
        _CACHE["nc"] = build_program()
    nc = _CACHE["nc"]
    in_maps = prep_inputs(inputs)
    res = run_bass_kernel_spmd(nc, in_maps, core_ids=list(range(N_CORES)))
    return assemble_output(res.results)
